# revision 6
# baseline (speedup 1.0000x reference)
"""LinearAttention Trainium2 kernel: data-parallel over batch on 8 NeuronCores.

Reference computation per batch b (C=256 channels, L=4096 seq, H=8 heads, D=64):
  qkv = w_qkv @ x[b]                    # (1536, L)
  q, k, v = split(qkv)                  # each (512, L), rows = (head, dim)
  k = softmax(k, axis=L)
  ctx[h] = k[h] @ v[h].T                # (64, 64)
  out[h] = ctx[h].T @ q[h]              # (64, L)
  y[b] = w_out @ concat(out) + b_out    # (256, L)

Per-core layout choices:
  - K^T, V^T computed with L on partitions (lhsT = x chunk, rhs = w^T) so the
    context matmul contracts over L on the TensorEngine.
  - softmax denominator comes for free: V^T tiles carry a 129th column of
    ones, so the context matmul's column 128 accumulates sum_l exp(k).
    exp() is applied unshifted (inputs are N(0,1)-scaled; max |k| ~ 5, safe
    in f32/bf16).
  - context matrices for head pairs are packed block-diagonally into a
    128x128 lhsT so the second attention matmul runs at full PE width.
  - all TensorE compute in bf16 (f32 PSUM accumulation).
"""

import numpy as np

B, C, L = 16, 256, 4096
HID = 512
N_CORES = 8
NB = B // N_CORES  # batches per core
CC = C // 128  # contraction chunks for the input projections (2)
LP = L // 128  # l-tiles with l on partitions (32)
LT = L // 512  # l-tiles of 512 for moving-dim matmuls (8)
PR = HID // 128  # head-pairs (4): each 128-wide chunk = 2 heads of 64

_CACHE = {}


def _build(reps=1):
    from concourse import bacc, mybir, tile
    import concourse.bass as bass

    bf16 = mybir.dt.bfloat16
    f32 = mybir.dt.float32
    Exp = mybir.ActivationFunctionType.Exp
    Copy = mybir.ActivationFunctionType.Copy
    Ident = mybir.ActivationFunctionType.Identity

    nc = bacc.Bacc(
        "TRN2",
        target_bir_lowering=False,
        debug=False,
        enable_asserts=False,
        num_devices=N_CORES,
    )

    x_d = nc.dram_tensor("x", [NB, CC, 128, L], bf16, kind="ExternalInput")
    wq_d = nc.dram_tensor("wq_t", [CC, 128, HID], bf16, kind="ExternalInput")
    wk_d = nc.dram_tensor("wk_t", [CC, 128, HID], bf16, kind="ExternalInput")
    wv_d = nc.dram_tensor("wv_t", [CC, 128, HID], bf16, kind="ExternalInput")
    wo_d = nc.dram_tensor("wo_t", [PR, 128, C], bf16, kind="ExternalInput")
    bb_d = nc.dram_tensor("bb", [128, 2], f32, kind="ExternalInput")
    out_d = nc.dram_tensor("out", [NB, 2, 128, L], f32, kind="ExternalOutput")

    with tile.TileContext(nc) as tc:
        with (
            tc.tile_pool(name="const", bufs=1) as const,
            tc.tile_pool(name="xp", bufs=2) as xp,
            tc.tile_pool(name="big", bufs=1) as big,
            tc.tile_pool(name="small", bufs=2) as small,
            tc.tile_pool(name="ps_mm", bufs=4, space="PSUM") as ps_mm,
            tc.tile_pool(name="ps_ctx", bufs=4, space="PSUM") as ps_ctx,
        ):
            wq = const.tile([128, CC, HID], bf16)
            wk = const.tile([128, CC, HID], bf16)
            wv = const.tile([128, CC, HID], bf16)
            wo = const.tile([128, PR, C], bf16)
            bb = const.tile([128, 2], f32)
            ctx_bd = const.tile([128, PR, 128], bf16)

            for cc in range(CC):
                nc.sync.dma_start(wq[:, cc, :], wq_d[cc])
                nc.sync.dma_start(wk[:, cc, :], wk_d[cc])
                nc.sync.dma_start(wv[:, cc, :], wv_d[cc])
            for pr in range(PR):
                nc.sync.dma_start(wo[:, pr, :], wo_d[pr])
            nc.sync.dma_start(bb[:], bb_d[:])
            nc.gpsimd.memset(ctx_bd[:], 0.0)

            for rep in range(reps):
              for bi in range(NB):
                xt = xp.tile([128, CC, L], bf16)
                for cc in range(CC):
                    nc.sync.dma_start(xt[:, cc, :], x_d[bi, cc])

                expkt = big.tile([128, LP, HID], bf16, tag="expkt")
                vt = big.tile([128, LP, PR, 129], bf16, tag="vt")
                ostg = big.tile([128, 2, L], f32, tag="ostg")
                nc.gpsimd.memset(vt[:, :, :, 128], 1.0)

                # K^T and V^T projections: l on partitions, channels on free.
                for lp in range(LP):
                    psk = ps_mm.tile([128, HID], f32, tag="mm")
                    for cc in range(CC):
                        nc.tensor.matmul(
                            psk[:],
                            xt[:, cc, lp * 128 : (lp + 1) * 128],
                            wk[:, cc, :],
                            start=(cc == 0),
                            stop=(cc == CC - 1),
                        )
                    nc.scalar.activation(expkt[:, lp, :], psk[:], Exp)
                    psv = ps_mm.tile([128, HID], f32, tag="mm")
                    for cc in range(CC):
                        nc.tensor.matmul(
                            psv[:],
                            xt[:, cc, lp * 128 : (lp + 1) * 128],
                            wv[:, cc, :],
                            start=(cc == 0),
                            stop=(cc == CC - 1),
                        )
                    for pr in range(PR):
                        nc.vector.tensor_copy(
                            vt[:, lp, pr, 0:128], psv[:, pr * 128 : (pr + 1) * 128]
                        )

                # context (+ denominator in column 128) per head-pair.
                ctx_p = [
                    ps_ctx.tile([128, 129], f32, tag="ctx", name=f"ctx_{bi}_{pr}")
                    for pr in range(PR)
                ]
                for pr in range(PR):
                    for lp in range(LP):
                        nc.tensor.matmul(
                            ctx_p[pr][:],
                            expkt[:, lp, pr * 128 : (pr + 1) * 128],
                            vt[:, lp, pr, :],
                            start=(lp == 0),
                            stop=(lp == LP - 1),
                        )
                inv_den = small.tile([128, PR], f32, tag="invden")
                for pr in range(PR):
                    nc.vector.reciprocal(inv_den[:, pr : pr + 1], ctx_p[pr][:, 128:129])
                    # block-diagonal packing: normalized per-head 64x64 blocks.
                    nc.scalar.activation(
                        ctx_bd[0:64, pr, 0:64],
                        ctx_p[pr][0:64, 0:64],
                        Copy,
                        scale=inv_den[0:64, pr : pr + 1],
                    )
                    nc.scalar.activation(
                        ctx_bd[64:128, pr, 64:128],
                        ctx_p[pr][64:128, 64:128],
                        Copy,
                        scale=inv_den[64:128, pr : pr + 1],
                    )

                # Q projection + attention out + output projection, per l-chunk.
                for lt in range(LT):
                    qt = small.tile([128, PR, 512], bf16, tag="qt")
                    for oc in range(PR):
                        psq = ps_mm.tile([128, 512], f32, tag="mm")
                        for cc in range(CC):
                            nc.tensor.matmul(
                                psq[:],
                                wq[:, cc, oc * 128 : (oc + 1) * 128],
                                xt[:, cc, lt * 512 : (lt + 1) * 512],
                                start=(cc == 0),
                                stop=(cc == CC - 1),
                            )
                        nc.vector.tensor_copy(qt[:, oc, :], psq[:])
                    at = small.tile([128, PR, 512], bf16, tag="at")
                    for pr in range(PR):
                        pso = ps_mm.tile([128, 512], f32, tag="mm")
                        nc.tensor.matmul(
                            pso[:], ctx_bd[:, pr, :], qt[:, pr, :], start=True, stop=True
                        )
                        nc.vector.tensor_copy(at[:, pr, :], pso[:])
                    for oc2 in range(2):
                        psf = ps_mm.tile([128, 512], f32, tag="mm")
                        for ch in range(PR):
                            nc.tensor.matmul(
                                psf[:],
                                wo[:, ch, oc2 * 128 : (oc2 + 1) * 128],
                                at[:, ch, :],
                                start=(ch == 0),
                                stop=(ch == PR - 1),
                            )
                        nc.scalar.activation(
                            ostg[:, oc2, lt * 512 : (lt + 1) * 512],
                            psf[:],
                            Ident,
                            bias=bb[:, oc2 : oc2 + 1],
                        )
                for oc2 in range(2):
                    nc.sync.dma_start(out_d[bi, oc2], ostg[:, oc2, :])

    nc.compile()
    return nc


def _get_nc():
    if "nc" not in _CACHE:
        _CACHE["nc"] = _build()
    return _CACHE["nc"]


def _prep_in_maps(x, w_qkv, w_out, b_out):
    import ml_dtypes

    bf16 = ml_dtypes.bfloat16
    wq_t = np.ascontiguousarray(w_qkv[0:512].T).reshape(CC, 128, HID).astype(bf16)
    wk_t = np.ascontiguousarray(w_qkv[512:1024].T).reshape(CC, 128, HID).astype(bf16)
    wv_t = np.ascontiguousarray(w_qkv[1024:1536].T).reshape(CC, 128, HID).astype(bf16)
    wo_t = np.ascontiguousarray(w_out.T).reshape(PR, 128, C).astype(bf16)
    bb = np.ascontiguousarray(b_out.reshape(2, 128).T).astype(np.float32)
    in_maps = []
    for c in range(N_CORES):
        xs = x[c * NB : (c + 1) * NB].reshape(NB, CC, 128, L).astype(bf16)
        in_maps.append(
            {
                "x": np.ascontiguousarray(xs),
                "wq_t": wq_t,
                "wk_t": wk_t,
                "wv_t": wv_t,
                "wo_t": wo_t,
                "bb": bb,
            }
        )
    return in_maps


def kernel(x, w_qkv, w_out, b_out):
    from concourse.bass_utils import run_bass_kernel_spmd

    nc = _get_nc()
    in_maps = _prep_in_maps(
        np.asarray(x, dtype=np.float32),
        np.asarray(w_qkv, dtype=np.float32),
        np.asarray(w_out, dtype=np.float32),
        np.asarray(b_out, dtype=np.float32),
    )
    res = run_bass_kernel_spmd(nc, in_maps, core_ids=list(range(N_CORES)))
    out = np.concatenate(
        [res.results[c]["out"].reshape(NB, C, L) for c in range(N_CORES)], axis=0
    )
    return out.astype(np.float32)


# revision 7
# speedup vs baseline: 1.3048x; 1.3048x over previous
"""LinearAttention Trainium2 kernel: data-parallel over batch on 8 NeuronCores.

Reference computation per batch b (C=256 channels, L=4096 seq, H=8 heads, D=64):
  qkv = w_qkv @ x[b]                    # (1536, L)
  q, k, v = split(qkv)                  # each (512, L), rows = (head, dim)
  k = softmax(k, axis=L)
  ctx[h] = k[h] @ v[h].T                # (64, 64)
  out[h] = ctx[h].T @ q[h]              # (64, L)
  y[b] = w_out @ concat(out) + b_out    # (256, L)

Per-core design (2 batches/core):
  - K^T, V^T computed with L on partitions (lhsT = x chunk, rhs = w^T) so the
    context matmul contracts over L on the TensorEngine.
  - context computed TRANSPOSED per head-pair: ctxT[e,d] = sum_l v[e,l]exp(k[d,l])
    (lhsT = v^T chunk, rhs = expk^T chunk), cross-head quadrants discarded via
    a zeroed block-diagonal SBUF tile.
  - w_out is folded into the context on the PE: McT[d,o] = sum_e ctxT[e,d]wo[e,o],
    which removes the separate attention-out matmul and its PSUM->SBUF copies.
    The softmax denominator (row matmul with a ones lhsT, then 4 tiny PE
    transposes) is applied as a per-partition ACT scale on the McT copy.
  - final: y = McT.T @ q + b, contracting the 512 q-channels in 4 chunks.
  - exp() applied unshifted (inputs are N(0,1)-scaled; max |k| ~ 5, safe in f32).
  - all TensorE compute in bf16 (f32 PSUM accumulation).
"""

import numpy as np

B, C, L = 16, 256, 4096
HID = 512
N_CORES = 8
NB = B // N_CORES  # batches per core
CC = C // 128  # contraction chunks for the input projections (2)
LP = L // 128  # l-tiles with l on partitions (32)
LT = L // 512  # l-tiles of 512 for moving-dim matmuls (8)
PR = HID // 128  # head-pairs (4): each 128-wide chunk = 2 heads of 64

_CACHE = {}


def _build(reps=1):
    from concourse import bacc, mybir, tile
    import concourse.bass as bass

    bf16 = mybir.dt.bfloat16
    f32 = mybir.dt.float32
    Exp = mybir.ActivationFunctionType.Exp
    Copy = mybir.ActivationFunctionType.Copy
    Ident = mybir.ActivationFunctionType.Identity

    nc = bacc.Bacc(
        "TRN2",
        target_bir_lowering=False,
        debug=False,
        enable_asserts=False,
        num_devices=N_CORES,
    )

    x_d = nc.dram_tensor("x", [NB, CC, 128, L], bf16, kind="ExternalInput")
    wq_d = nc.dram_tensor("wq_t", [CC, 128, HID], bf16, kind="ExternalInput")
    wk_d = nc.dram_tensor("wk_t", [CC, 128, HID], bf16, kind="ExternalInput")
    wv_d = nc.dram_tensor("wv_t", [CC, 128, HID], bf16, kind="ExternalInput")
    wo_d = nc.dram_tensor("wo_t", [PR, 128, C], bf16, kind="ExternalInput")
    bb_d = nc.dram_tensor("bb", [128, 2], f32, kind="ExternalInput")
    out_d = nc.dram_tensor("out", [NB, 2, 128, L], f32, kind="ExternalOutput")

    with tile.TileContext(nc) as tc:
        with (
            tc.tile_pool(name="const", bufs=1) as const,
            tc.tile_pool(name="xp", bufs=2) as xp,
            tc.tile_pool(name="big", bufs=1) as big,
            tc.tile_pool(name="small", bufs=2) as small,
            tc.tile_pool(name="ps_mm", bufs=3, space="PSUM") as ps_mm,
            tc.tile_pool(name="ps_ctx", bufs=4, space="PSUM") as ps_ctx,
            tc.tile_pool(name="ps_den", bufs=1, space="PSUM") as ps_den,
        ):
            wq = const.tile([128, CC, HID], bf16)
            wk = const.tile([128, CC, HID], bf16)
            wv = const.tile([128, CC, HID], bf16)
            wo = const.tile([128, PR, C], bf16)
            bb = const.tile([128, 2], f32)
            ones_col = const.tile([128, 1], bf16)
            id11 = const.tile([1, 1], f32)
            ctxt_sb = const.tile([128, PR, 128], bf16)

            for cc in range(CC):
                nc.sync.dma_start(wq[:, cc, :], wq_d[cc])
                nc.sync.dma_start(wk[:, cc, :], wk_d[cc])
                nc.sync.dma_start(wv[:, cc, :], wv_d[cc])
            for pr in range(PR):
                nc.sync.dma_start(wo[:, pr, :], wo_d[pr])
            nc.sync.dma_start(bb[:], bb_d[:])
            nc.gpsimd.memset(ones_col[:], 1.0)
            nc.gpsimd.memset(id11[:], 1.0)
            nc.gpsimd.memset(ctxt_sb[:], 0.0)

            for rep in range(reps):
              for bi in range(NB):
                xt = xp.tile([128, CC, L], bf16)
                for cc in range(CC):
                    nc.sync.dma_start(xt[:, cc, :], x_d[bi, cc])

                expkt = big.tile([128, LP, HID], bf16, tag="expkt")
                vt = big.tile([128, LP, HID], bf16, tag="vt")
                ostg = big.tile([128, 2, L], f32, tag="ostg")

                # K^T and V^T projections: l on partitions, channels on free.
                for lp in range(LP):
                    psk = ps_mm.tile([128, HID], f32, tag="mm")
                    psv = ps_mm.tile([128, HID], f32, tag="mm")
                    for cc in range(CC):
                        nc.tensor.matmul(
                            psk[:],
                            xt[:, cc, lp * 128 : (lp + 1) * 128],
                            wk[:, cc, :],
                            start=(cc == 0),
                            stop=(cc == CC - 1),
                        )
                    for cc in range(CC):
                        nc.tensor.matmul(
                            psv[:],
                            xt[:, cc, lp * 128 : (lp + 1) * 128],
                            wv[:, cc, :],
                            start=(cc == 0),
                            stop=(cc == CC - 1),
                        )
                    nc.scalar.activation(expkt[:, lp, :], psk[:], Exp)
                    nc.vector.tensor_copy(vt[:, lp, :], psv[:])

                # transposed context per head-pair, accumulated over l.
                ctx_p = [
                    ps_ctx.tile([128, 128], f32, tag="ctx", name=f"ctx_{rep}_{bi}_{pr}")
                    for pr in range(PR)
                ]
                for pr in range(PR):
                    for lp in range(LP):
                        nc.tensor.matmul(
                            ctx_p[pr][:],
                            vt[:, lp, pr * 128 : (pr + 1) * 128],
                            expkt[:, lp, pr * 128 : (pr + 1) * 128],
                            start=(lp == 0),
                            stop=(lp == LP - 1),
                        )
                # softmax denominators: row vector, then transpose to columns.
                den_ps = ps_den.tile([1, HID], f32, tag="den")
                for lp in range(LP):
                    nc.tensor.matmul(
                        den_ps[:],
                        ones_col[:],
                        expkt[:, lp, :],
                        start=(lp == 0),
                        stop=(lp == LP - 1),
                    )
                den_sb = small.tile([1, HID], f32, tag="densb")
                nc.vector.tensor_copy(den_sb[:], den_ps[:])
                tps = ps_mm.tile([128, PR], f32, tag="mm")
                for pr in range(PR):
                    nc.tensor.transpose(
                        tps[:, pr : pr + 1],
                        den_sb[0:1, pr * 128 : (pr + 1) * 128],
                        id11[:],
                    )
                inv_den = small.tile([128, PR], f32, tag="invden")
                nc.vector.reciprocal(inv_den[:], tps[:])

                # block-diagonal ctxT (cross-head quadrants stay zero).
                for pr in range(PR):
                    nc.vector.tensor_copy(
                        ctxt_sb[0:64, pr, 0:64], ctx_p[pr][0:64, 0:64]
                    )
                    nc.vector.tensor_copy(
                        ctxt_sb[64:128, pr, 64:128], ctx_p[pr][64:128, 64:128]
                    )
                # fold w_out into the context: McT[d, o], scaled by 1/den[d].
                mct = small.tile([128, PR, C], bf16, tag="mct")
                for pr in range(PR):
                    mc_ps = ps_mm.tile([128, C], f32, tag="mm")
                    nc.tensor.matmul(
                        mc_ps[:], ctxt_sb[:, pr, :], wo[:, pr, :], start=True, stop=True
                    )
                    nc.scalar.activation(
                        mct[:, pr, :], mc_ps[:], Copy, scale=inv_den[:, pr : pr + 1]
                    )

                # Q projection + fused output projection, per l-chunk of 512.
                for lt in range(LT):
                    qt = small.tile([128, PR, 512], bf16, tag="qt")
                    for oc in range(PR):
                        psq = ps_mm.tile([128, 512], f32, tag="mm")
                        for cc in range(CC):
                            nc.tensor.matmul(
                                psq[:],
                                wq[:, cc, oc * 128 : (oc + 1) * 128],
                                xt[:, cc, lt * 512 : (lt + 1) * 512],
                                start=(cc == 0),
                                stop=(cc == CC - 1),
                            )
                        nc.vector.tensor_copy(qt[:, oc, :], psq[:])
                    for oc2 in range(2):
                        psf = ps_mm.tile([128, 512], f32, tag="mm")
                        for pr in range(PR):
                            nc.tensor.matmul(
                                psf[:],
                                mct[:, pr, oc2 * 128 : (oc2 + 1) * 128],
                                qt[:, pr, :],
                                start=(pr == 0),
                                stop=(pr == PR - 1),
                            )
                        nc.scalar.activation(
                            ostg[:, oc2, lt * 512 : (lt + 1) * 512],
                            psf[:],
                            Ident,
                            bias=bb[:, oc2 : oc2 + 1],
                        )
                for oc2 in range(2):
                    nc.sync.dma_start(out_d[bi, oc2], ostg[:, oc2, :])

    nc.compile()
    return nc


def _get_nc():
    if "nc" not in _CACHE:
        _CACHE["nc"] = _build()
    return _CACHE["nc"]


def _prep_in_maps(x, w_qkv, w_out, b_out):
    import ml_dtypes

    bf16 = ml_dtypes.bfloat16
    wq_t = np.ascontiguousarray(w_qkv[0:512].T).reshape(CC, 128, HID).astype(bf16)
    wk_t = np.ascontiguousarray(w_qkv[512:1024].T).reshape(CC, 128, HID).astype(bf16)
    wv_t = np.ascontiguousarray(w_qkv[1024:1536].T).reshape(CC, 128, HID).astype(bf16)
    wo_t = np.ascontiguousarray(w_out.T).reshape(PR, 128, C).astype(bf16)
    bb = np.ascontiguousarray(b_out.reshape(2, 128).T).astype(np.float32)
    in_maps = []
    for c in range(N_CORES):
        xs = x[c * NB : (c + 1) * NB].reshape(NB, CC, 128, L).astype(bf16)
        in_maps.append(
            {
                "x": np.ascontiguousarray(xs),
                "wq_t": wq_t,
                "wk_t": wk_t,
                "wv_t": wv_t,
                "wo_t": wo_t,
                "bb": bb,
            }
        )
    return in_maps


def kernel(x, w_qkv, w_out, b_out):
    from concourse.bass_utils import run_bass_kernel_spmd

    nc = _get_nc()
    in_maps = _prep_in_maps(
        np.asarray(x, dtype=np.float32),
        np.asarray(w_qkv, dtype=np.float32),
        np.asarray(w_out, dtype=np.float32),
        np.asarray(b_out, dtype=np.float32),
    )
    res = run_bass_kernel_spmd(nc, in_maps, core_ids=list(range(N_CORES)))
    out = np.concatenate(
        [res.results[c]["out"].reshape(NB, C, L) for c in range(N_CORES)], axis=0
    )
    return out.astype(np.float32)


# revision 15
# speedup vs baseline: 1.8214x; 1.3959x over previous
"""LinearAttention Trainium2 kernel: data-parallel over batch on 8 NeuronCores.

Reference computation per batch b (C=256 channels, L=4096 seq, H=8 heads, D=64):
  qkv = w_qkv @ x[b]                    # (1536, L)
  q, k, v = split(qkv)                  # each (512, L), rows = (head, dim)
  k = softmax(k, axis=L)
  ctx[h] = k[h] @ v[h].T                # (64, 64)
  out[h] = ctx[h].T @ q[h]              # (64, L)
  y[b] = w_out @ concat(out) + b_out    # (256, L)

Per-core design (2 batches/core):
  - K^T, V^T computed with L on partitions (lhsT = x chunk, rhs = w^T) so the
    context matmul contracts over L on the TensorEngine.
  - context computed TRANSPOSED per head-pair: ctxT[e,d] = sum_l v[e,l]exp(k[d,l])
    (lhsT = v^T chunk, rhs = expk^T chunk), cross-head quadrants discarded via
    a zeroed block-diagonal SBUF tile.
  - w_out is folded into the context on the PE: McT[d,o] = sum_e ctxT[e,d]wo[e,o],
    which removes the separate attention-out matmul and its PSUM->SBUF copies.
    The softmax denominator (row matmul with a ones lhsT, then 4 tiny PE
    transposes) is applied as a per-partition ACT scale on the McT copy.
  - final: y = McT.T @ q + b, contracting the 512 q-channels in 4 chunks.
  - exp() applied unshifted (inputs are N(0,1)-scaled; max |k| ~ 5, safe in f32).
  - all TensorE compute in bf16 (f32 PSUM accumulation).
"""

import numpy as np

B, C, L = 16, 256, 4096
HID = 512
N_CORES = 8
NB = B // N_CORES  # batches per core
CC = C // 128  # contraction chunks for the input projections (2)
LP = L // 128  # l-tiles with l on partitions (32)
LT = L // 512  # l-tiles of 512 for moving-dim matmuls (8)
PR = HID // 128  # head-pairs (4): each 128-wide chunk = 2 heads of 64

_CACHE = {}


def _build(reps=1):
    from concourse import bacc, mybir, tile
    import concourse.bass as bass

    bf16 = mybir.dt.bfloat16
    f32 = mybir.dt.float32
    Exp = mybir.ActivationFunctionType.Exp
    Copy = mybir.ActivationFunctionType.Copy
    Ident = mybir.ActivationFunctionType.Identity

    nc = bacc.Bacc(
        "TRN2",
        target_bir_lowering=False,
        debug=False,
        enable_asserts=False,
        num_devices=N_CORES,
    )

    x_d = nc.dram_tensor("x", [NB, CC, 128, L], bf16, kind="ExternalInput")
    wq_d = nc.dram_tensor("wq_t", [CC, 128, HID], bf16, kind="ExternalInput")
    wk_d = nc.dram_tensor("wk_t", [CC, 128, HID], bf16, kind="ExternalInput")
    wv_d = nc.dram_tensor("wv_t", [CC, 128, HID], bf16, kind="ExternalInput")
    wo_d = nc.dram_tensor("wo_t", [PR, 128, C], bf16, kind="ExternalInput")
    bb_d = nc.dram_tensor("bb", [128, 2], f32, kind="ExternalInput")
    out_d = nc.dram_tensor("out", [NB, 2, 128, L], f32, kind="ExternalOutput")

    with tile.TileContext(nc) as tc:
        with (
            tc.tile_pool(name="const", bufs=1) as const,
            tc.tile_pool(name="xp", bufs=2) as xp,
            tc.tile_pool(name="big", bufs=1) as big,
            tc.tile_pool(name="small", bufs=2) as small,
            tc.tile_pool(name="qtp", bufs=4) as qtp,
            tc.tile_pool(name="ostp", bufs=3) as ostp,
            tc.tile_pool(name="ps_mm", bufs=3, space="PSUM") as ps_mm,
            tc.tile_pool(name="ps_ctx", bufs=4, space="PSUM") as ps_ctx,
            tc.tile_pool(name="ps_den", bufs=1, space="PSUM") as ps_den,
        ):
            wq = const.tile([128, CC, HID], bf16)
            wk = const.tile([128, CC, HID], bf16)
            wv = const.tile([128, CC, HID], bf16)
            wo = const.tile([128, PR, C], bf16)
            bb = const.tile([128, 2], f32)
            ones_col = const.tile([128, 1], bf16)
            id11 = const.tile([1, 1], f32)
            ctxt_sb = const.tile([128, PR, 128], bf16)

            for cc in range(CC):
                nc.sync.dma_start(wq[:, cc, :], wq_d[cc])
                nc.sync.dma_start(wk[:, cc, :], wk_d[cc])
                nc.sync.dma_start(wv[:, cc, :], wv_d[cc])
            for pr in range(PR):
                nc.sync.dma_start(wo[:, pr, :], wo_d[pr])
            nc.sync.dma_start(bb[:], bb_d[:])
            nc.gpsimd.memset(ones_col[:], 1.0)
            nc.gpsimd.memset(id11[:], 1.0)
            nc.gpsimd.memset(ctxt_sb[:], 0.0)

            for rep in range(reps):
              for bi in range(NB):
                xt = xp.tile([128, CC, L], bf16)
                for cc in range(CC):
                    nc.sync.dma_start(xt[:, cc, :], x_d[bi, cc])

                expkt = big.tile([128, LP, HID], bf16, tag="expkt")
                vt = big.tile([128, LP, HID], bf16, tag="vt")

                # K^T / V^T projections fused with the transposed-context and
                # denominator accumulations: PE streams without phase breaks.
                ctx_p = [
                    ps_ctx.tile([128, 128], f32, tag="ctx", name=f"ctx_{rep}_{bi}_{g}")
                    for g in range(PR)
                ]
                den_ps = ps_den.tile([1, HID], f32, tag="den")
                for lp in range(LP):
                    psk = ps_mm.tile([128, HID], f32, tag="mm")
                    psv = ps_mm.tile([128, HID], f32, tag="mm")
                    for cc in range(CC):
                        nc.tensor.matmul(
                            psk[:],
                            xt[:, cc, lp * 128 : (lp + 1) * 128],
                            wk[:, cc, :],
                            start=(cc == 0),
                            stop=(cc == CC - 1),
                        )
                    for cc in range(CC):
                        nc.tensor.matmul(
                            psv[:],
                            xt[:, cc, lp * 128 : (lp + 1) * 128],
                            wv[:, cc, :],
                            start=(cc == 0),
                            stop=(cc == CC - 1),
                        )
                    nc.scalar.activation(expkt[:, lp, :], psk[:], Exp)
                    nc.vector.tensor_copy(vt[:, lp, :], psv[:])
                    for pr in range(PR):
                        nc.tensor.matmul(
                            ctx_p[pr][:],
                            vt[:, lp, pr * 128 : (pr + 1) * 128],
                            expkt[:, lp, pr * 128 : (pr + 1) * 128],
                            start=(lp == 0),
                            stop=(lp == LP - 1),
                        )
                    nc.tensor.matmul(
                        den_ps[:],
                        ones_col[:],
                        expkt[:, lp, :],
                        start=(lp == 0),
                        stop=(lp == LP - 1),
                        skip_group_check=True,
                    )
                den_sb = small.tile([1, HID], f32, tag="densb")
                nc.vector.tensor_copy(den_sb[:], den_ps[:])
                tps = ps_mm.tile([128, PR], f32, tag="mm")
                for pr in range(PR):
                    nc.tensor.transpose(
                        tps[:, pr : pr + 1],
                        den_sb[0:1, pr * 128 : (pr + 1) * 128],
                        id11[:],
                    )
                inv_den = small.tile([128, PR], f32, tag="invden")
                nc.vector.reciprocal(inv_den[:], tps[:])

                # block-diagonal ctxT (cross-head quadrants stay zero).
                for pr in range(PR):
                    nc.vector.tensor_copy(
                        ctxt_sb[0:64, pr, 0:64], ctx_p[pr][0:64, 0:64]
                    )
                    nc.vector.tensor_copy(
                        ctxt_sb[64:128, pr, 64:128], ctx_p[pr][64:128, 64:128]
                    )
                # fold w_out into the context: McT[d, o], scaled by 1/den[d].
                mct = small.tile([128, PR, C], bf16, tag="mct")
                for pr in range(PR):
                    mc_ps = ps_mm.tile([128, C], f32, tag="mm")
                    nc.tensor.matmul(
                        mc_ps[:], ctxt_sb[:, pr, :], wo[:, pr, :], start=True, stop=True
                    )
                    nc.scalar.activation(
                        mct[:, pr, :], mc_ps[:], Copy, scale=inv_den[:, pr : pr + 1]
                    )

                # Q projection + fused output projection, per l-chunk of 512.
                for lt in range(LT):
                    qt = qtp.tile([128, PR, 512], bf16, tag="qt")
                    for oc in range(PR):
                        psq = ps_mm.tile([128, 512], f32, tag="mm")
                        for cc in range(CC):
                            nc.tensor.matmul(
                                psq[:],
                                wq[:, cc, oc * 128 : (oc + 1) * 128],
                                xt[:, cc, lt * 512 : (lt + 1) * 512],
                                start=(cc == 0),
                                stop=(cc == CC - 1),
                            )
                        nc.vector.tensor_copy(qt[:, oc, :], psq[:])
                    ostg = ostp.tile([128, 2, 512], f32, tag="ostg")
                    for oc2 in range(2):
                        psf = ps_mm.tile([128, 512], f32, tag="mm")
                        for pr in range(PR):
                            nc.tensor.matmul(
                                psf[:],
                                mct[:, pr, oc2 * 128 : (oc2 + 1) * 128],
                                qt[:, pr, :],
                                start=(pr == 0),
                                stop=(pr == PR - 1),
                            )
                        nc.scalar.activation(
                            ostg[:, oc2, :],
                            psf[:],
                            Ident,
                            bias=bb[:, oc2 : oc2 + 1],
                        )
                        nc.sync.dma_start(
                            out_d[bi, oc2, :, lt * 512 : (lt + 1) * 512],
                            ostg[:, oc2, :],
                        )

    nc.compile()
    return nc


def _get_nc():
    if "nc" not in _CACHE:
        _CACHE["nc"] = _build()
    return _CACHE["nc"]


def _prep_in_maps(x, w_qkv, w_out, b_out):
    import ml_dtypes

    bf16 = ml_dtypes.bfloat16
    wq_t = np.ascontiguousarray(w_qkv[0:512].T).reshape(CC, 128, HID).astype(bf16)
    wk_t = np.ascontiguousarray(w_qkv[512:1024].T).reshape(CC, 128, HID).astype(bf16)
    wv_t = np.ascontiguousarray(w_qkv[1024:1536].T).reshape(CC, 128, HID).astype(bf16)
    wo_t = np.ascontiguousarray(w_out.T).reshape(PR, 128, C).astype(bf16)
    bb = np.ascontiguousarray(b_out.reshape(2, 128).T).astype(np.float32)
    in_maps = []
    for c in range(N_CORES):
        xs = x[c * NB : (c + 1) * NB].reshape(NB, CC, 128, L).astype(bf16)
        in_maps.append(
            {
                "x": np.ascontiguousarray(xs),
                "wq_t": wq_t,
                "wk_t": wk_t,
                "wv_t": wv_t,
                "wo_t": wo_t,
                "bb": bb,
            }
        )
    return in_maps


def kernel(x, w_qkv, w_out, b_out):
    from concourse.bass_utils import run_bass_kernel_spmd

    nc = _get_nc()
    in_maps = _prep_in_maps(
        np.asarray(x, dtype=np.float32),
        np.asarray(w_qkv, dtype=np.float32),
        np.asarray(w_out, dtype=np.float32),
        np.asarray(b_out, dtype=np.float32),
    )
    res = run_bass_kernel_spmd(nc, in_maps, core_ids=list(range(N_CORES)))
    out = np.concatenate(
        [res.results[c]["out"].reshape(NB, C, L) for c in range(N_CORES)], axis=0
    )
    return out.astype(np.float32)
